# revision 13
# baseline (speedup 1.0000x reference)
"""Trainium2 Bass kernel for the shifted-window attention block
(nn_Block_6373731467375), SPMD over 8 NeuronCores, data-parallel over batch.

Per core: 2 batch elements. Pass A computes the attention branch in rolled
window space (LN1 folded into qkv weights, dual-S softmax: S token-major for
denominators, S feature-major for the AV matmul), writes the scaled branch
output to a DRAM scratch in original token order. Pass B adds the residual,
applies LN2 (folded into fc1), runs the MLP and writes the final output.
"""

import numpy as np
import ml_dtypes

BF = ml_dtypes.bfloat16

DIM, H, HD, WS, SHIFT, NPATCH, MLP, EPS = 768, 12, 64, 128, 64, 128, 3072, 1e-5
B, N = 16, 2000
NCORES = 8
BL = B // NCORES          # batch elems per core
TOK = BL * N              # 4000
NW = 16                   # rolled 128-token tiles (=windows) per batch elem
NG = 4                    # groups of 4 tiles (512 tokens)
CC = DIM // 128           # 6 contraction chunks
JB = MLP // 128           # 24 hidden blocks
MAGIC = 0x5F3759DF

_CACHE = {}


# ---------------------------------------------------------------------------
# device kernel builder
# ---------------------------------------------------------------------------

def _fix_multi_waits(nc, mybir):
    """This walrus build rejects >1 sync-wait per instruction; hoist extra
    waits onto dedicated NOPs inserted just before, on the same engine."""
    n = 0
    for blk in nc.main_func.blocks:
        new_insts = []
        changed = False
        for ins in blk.instructions:
            si = ins.sync_info
            if si is not None and si.on_wait and len(si.on_wait) > 1:
                waits = list(si.on_wait)
                for w in waits[:-1]:
                    n += 1
                    nop = mybir.InstNoOp(
                        name=f"{ins.name}-sw{n}",
                        engine=ins.engine,
                        ins=[],
                        outs=[],
                        bass_nofuse=True,
                        sync_info=mybir.SyncInfo(on_wait=[w], on_update=[]),
                    )
                    new_insts.append(nop)
                si.on_wait = waits[-1:]
                changed = True
            new_insts.append(ins)
        if changed:
            blk.instructions = new_insts
    return n


def _build(fix_waits=True, passes=('A','B')):
    import concourse.bass as bass
    import concourse.mybir as mybir
    from contextlib import ExitStack

    f32 = mybir.dt.float32
    bf16 = mybir.dt.bfloat16
    u32 = mybir.dt.uint32
    AX = mybir.AxisListType
    OP = mybir.AluOpType
    AF = mybir.ActivationFunctionType

    from concourse.tile import TileContext

    nc = bass.Bass()
    p = {}
    p["xs"] = nc.declare_dram_parameter("xs", [TOK, DIM], f32, isOutput=False)
    p["wqk"] = nc.declare_dram_parameter("wqk", [128, CC, 2 * DIM], bf16, isOutput=False)
    p["wv"] = nc.declare_dram_parameter("wv", [128, CC, DIM], bf16, isOutput=False)
    p["wproj"] = nc.declare_dram_parameter("wproj", [128, CC, DIM], bf16, isOutput=False)
    p["wfc1"] = nc.declare_dram_parameter("wfc1", [128, CC, MLP], bf16, isOutput=False)
    p["wfc2"] = nc.declare_dram_parameter("wfc2", [128, JB, DIM], bf16, isOutput=False)
    p["bqk"] = nc.declare_dram_parameter("bqk", [128, 12], f32, isOutput=False)
    p["bfc1"] = nc.declare_dram_parameter("bfc1", [128, JB], f32, isOutput=False)
    p["bproj"] = nc.declare_dram_parameter("bproj", [128, DIM], f32, isOutput=False)
    p["bfc2x"] = nc.declare_dram_parameter("bfc2x", [128, DIM], f32, isOutput=False)
    p["btok"] = nc.declare_dram_parameter("btok", [128, 2, 6, WS], bf16, isOutput=False)
    p["btokm"] = nc.declare_dram_parameter("btokm", [128, 2, 6, WS], bf16, isOutput=False)
    p["bfeat"] = nc.declare_dram_parameter("bfeat", [128, 2, 6, WS], bf16, isOutput=False)
    p["bfeatm"] = nc.declare_dram_parameter("bfeatm", [128, 2, 6, WS], bf16, isOutput=False)
    p["ident"] = nc.declare_dram_parameter("ident", [128, 128], bf16, isOutput=False)
    out_t = nc.declare_dram_parameter("out", [TOK, DIM], f32, isOutput=True)
    attn_scr = nc.dram_tensor("attn_scr", [TOK, DIM], bf16)

    with TileContext(nc) as tc, ExitStack() as ctx:
        cpool = ctx.enter_context(tc.tile_pool(name="consts", bufs=1))

        # resident constants
        sb = {}
        for name in ("bqk", "bfc1", "bproj", "bfc2x", "btok", "btokm",
                     "bfeat", "bfeatm", "ident"):
            t = cpool.tile(list(p[name].shape), p[name].dtype, tag=name)
            nc.sync.dma_start(out=t[:], in_=p[name][:])
            sb[name] = t
        magic = cpool.tile([128, 1], u32, tag="magic")
        nc.vector.memset(magic[:], MAGIC)

        wB = ctx.enter_context(tc.tile_pool(name="wB", bufs=1))
        wfc1 = wB.tile([128, CC, MLP], bf16)
        nc.sync.dma_start(out=wfc1[:], in_=p["wfc1"][:])
        wfc2 = wB.tile([128, JB, DIM], bf16)
        nc.sync.dma_start(out=wfc2[:], in_=p["wfc2"][:])

        # ---------------- helpers ----------------
        def newton_rsqrt(pool, var_view, rstdg, tagp):
            """rstdg[:, :NG] = rsqrt(var_view + eps) via 3 fp32 Newton steps."""
            vts = pool.tile([128, NG], f32, tag=tagp + "v")
            y = pool.tile([128, NG], f32, tag=tagp + "y")
            t1 = pool.tile([128, NG], f32, tag=tagp + "t")
            nc.vector.tensor_scalar_add(out=vts[:], in0=var_view, scalar1=EPS)
            nc.vector.tensor_scalar(
                out=y[:].bitcast(u32),
                in0=vts[:].bitcast(u32),
                scalar1=1,
                scalar2=None,
                op0=OP.logical_shift_right,
            )
            nc.vector.tensor_tensor(
                out=y[:].bitcast(u32),
                in0=magic[:].to_broadcast([128, NG]),
                in1=y[:].bitcast(u32),
                op=OP.subtract,
            )
            a, b = y, rstdg
            for _ in range(3):
                nc.vector.tensor_mul(out=t1[:], in0=a[:], in1=a[:])
                nc.vector.tensor_mul(out=t1[:], in0=t1[:], in1=vts[:])
                nc.vector.tensor_scalar(
                    out=t1[:], in0=t1[:], scalar1=-0.5, scalar2=1.5,
                    op0=OP.mult, op1=OP.add,
                )
                nc.vector.tensor_mul(out=b[:], in0=a[:], in1=t1[:])
                a, b = b, a
            assert a is rstdg  # odd iteration count lands in caller's tile

        def transpose6(pool, z_t, dst, tl):
            """z_t [128,768] bf16 -> dst[:, :, tl*128:(tl+1)*128] ([128,6,128])."""
            zT = pool.tile([128, DIM], bf16, tag="px")
            for cc in range(CC):
                nc.tensor.matmul(
                    zT[:, cc * 128 : (cc + 1) * 128],
                    z_t[:, cc * 128 : (cc + 1) * 128],
                    sb["ident"][:],
                    start=(cc == 0), stop=(cc == CC - 1),
                    is_transpose=True,
                )
            nc.vector.tensor_copy(
                out=dst[:, :, tl * 128 : (tl + 1) * 128],
                in_=zT[:].rearrange("p (c q) -> p c q", c=CC),
            )

        # =================== PASS A ===================
        if 'A' in passes:
         with tc.tile_pool(name="wA", bufs=1) as wA, \
             tc.tile_pool(name="pa", bufs=2) as pa, \
             tc.tile_pool(name="pa1", bufs=1) as pa1, \
             tc.tile_pool(name="pa3", bufs=3) as pa3, \
             tc.tile_pool(name="px", bufs=2, space="PSUM") as px, \
             tc.tile_pool(name="ps", bufs=2, space="PSUM") as ps:

            wqk = wA.tile([128, CC, 2 * DIM], bf16)
            nc.sync.dma_start(out=wqk[:], in_=p["wqk"][:])
            wv = wA.tile([128, CC, DIM], bf16)
            nc.sync.dma_start(out=wv[:], in_=p["wv"][:])
            wproj = wA.tile([128, CC, DIM], bf16)
            nc.sync.dma_start(out=wproj[:], in_=p["wproj"][:])

            for b in range(BL):
                x0 = b * N
                for g in range(NG):
                    hT = pa.tile([128, CC, 512], bf16, tag="hT")
                    mvg = pa.tile([128, NG, 2], f32, tag="mvg")
                    rstdg = pa.tile([128, NG], f32, tag="rstdg")
                    xG = pa.tile([128, NG, DIM], bf16, tag="xG")
                    for tl in range(4):
                        t = 4 * g + tl
                        if t < NW - 1:
                            nc.gpsimd.dma_start(
                                out=xG[:, tl, :],
                                in_=p["xs"][x0 + 64 + 128 * t : x0 + 64 + 128 * (t + 1)],
                            )
                        else:
                            nc.vector.memset(xG[:, tl, :], 0.0)
                            nc.gpsimd.dma_start(
                                out=xG[0:16, tl, :], in_=p["xs"][x0 + 1984 : x0 + 2000]
                            )
                            nc.gpsimd.dma_start(
                                out=xG[64:128, tl, :], in_=p["xs"][x0 : x0 + 64]
                            )
                        stats = pa3.tile([128, 3, 6], f32, tag="ln_stats")
                        for sg in range(3):
                            nc.vector.bn_stats(
                                out=stats[:, sg, :],
                                in_=xG[:, tl, sg * 256 : (sg + 1) * 256],
                            )
                        nc.vector.bn_aggr(out=mvg[:, tl, :], in_=stats[:])
                    newton_rsqrt(pa3, mvg[:, :, 1], rstdg, "nra")
                    for tl in range(4):
                        z_t = pa3.tile([128, DIM], bf16, tag="z_t")
                        nc.vector.tensor_scalar(
                            out=z_t[:], in0=xG[:, tl, :],
                            scalar1=mvg[:, tl, 0:1], scalar2=rstdg[:, tl : tl + 1],
                            op0=OP.subtract, op1=OP.mult,
                        )
                        transpose6(px, z_t, hT, tl)

                    # qkv for the group
                    QKT = pa.tile([128, 12, 512], bf16, tag="QKT")
                    for fb in range(12):
                        qk_ps = px.tile([128, 512], f32, tag="px")
                        for cc in range(CC):
                            nc.tensor.matmul(
                                qk_ps[:],
                                wqk[:, cc, fb * 128 : (fb + 1) * 128],
                                hT[:, cc, :],
                                start=(cc == 0), stop=(cc == CC - 1),
                            )
                        nc.vector.tensor_scalar(
                            out=QKT[:, fb, :], in0=qk_ps[:],
                            scalar1=sb["bqk"][:, fb : fb + 1],
                            scalar2=(0.125 if fb < 6 else 1.0),
                            op0=OP.add, op1=OP.mult,
                        )
                    VG = pa1.tile([128, 4, DIM], bf16, tag="VG")
                    for tl in range(4):
                        v_ps = px.tile([128, 2, 512], f32, tag="px")
                        for nh in range(2):
                            for cc in range(CC):
                                nc.tensor.matmul(
                                    v_ps[:, nh, 0:384],
                                    hT[:, cc, tl * 128 : (tl + 1) * 128],
                                    wv[:, cc, nh * 384 : (nh + 1) * 384],
                                    start=(cc == 0), stop=(cc == CC - 1),
                                )
                        nc.vector.tensor_copy(
                            out=VG[:, tl, :].rearrange("p (a n) -> p a n", a=2),
                            in_=v_ps[:, :, 0:384],
                        )

                    # windows
                    for tl in range(4):
                        t = 4 * g + tl
                        masked = t == NW - 1
                        btok_t = sb["btokm"] if masked else sb["btok"]
                        bfeat_t = sb["bfeatm"] if masked else sb["bfeat"]
                        qs = slice(tl * 128, (tl + 1) * 128)

                        # Even/odd heads target different PSUM banks: MMs with
                        # disjoint PE row-groups (base partition 0 vs 64) run
                        # concurrently, and concurrent writes to one PSUM bank
                        # hard-fault the device. Slot j: even i -> i//2 (bank
                        # 0), odd i -> 4 + i//2 (bank 1).
                        den = pa.tile([128, 4, 3], f32, tag="den")
                        rden = pa.tile([128, 4, 3], f32, tag="rden")
                        e_feat = []
                        for half in range(2):
                            hh = list(range(half * 6, half * 6 + 6))

                            def _smm(s_t, lhs_fb, rhs_fb):
                                for i, h in enumerate(hh):
                                    bp = (h % 2) * 64
                                    j = (i // 2) + 4 * (i % 2)
                                    nc.tensor.matmul(
                                        s_t[:, j, :],
                                        QKT[bp : bp + 64, lhs_fb + h // 2, qs],
                                        QKT[bp : bp + 64, rhs_fb + h // 2, qs],
                                        start=(i in (0, 1)), stop=(i in (4, 5)),
                                    )

                            s_tok = ps.tile([128, 8, 128], f32, tag="s")
                            _smm(s_tok, 0, 6)
                            E_tok = pa.tile([128, 8, 128], bf16, tag="E_tok")
                            for blk in range(2):
                                js = slice(4 * blk, 4 * blk + 3)
                                nc.scalar.activation(
                                    out=E_tok[:, js, :], in_=s_tok[:, js, :],
                                    func=AF.Exp,
                                )
                                nc.vector.tensor_mul(
                                    out=E_tok[:, js, :], in0=E_tok[:, js, :],
                                    in1=btok_t[:, half, 3 * blk : 3 * blk + 3, :],
                                )
                                nc.vector.tensor_reduce(
                                    out=den[:, half * 2 + blk, :],
                                    in_=E_tok[:, js, :],
                                    axis=AX.X, op=OP.add,
                                )

                            s_feat = ps.tile([128, 8, 128], f32, tag="s")
                            _smm(s_feat, 6, 0)
                            E_f = pa.tile([128, 8, 128], bf16, tag="E_feat")
                            for blk in range(2):
                                js = slice(4 * blk, 4 * blk + 3)
                                nc.scalar.activation(
                                    out=E_f[:, js, :], in_=s_feat[:, js, :],
                                    func=AF.Exp,
                                )
                                nc.vector.tensor_mul(
                                    out=E_f[:, js, :], in0=E_f[:, js, :],
                                    in1=bfeat_t[:, half, 3 * blk : 3 * blk + 3, :],
                                )
                            e_feat.append(E_f)
                        nc.vector.reciprocal(out=rden[:], in_=den[:])

                        O_ps = px.tile([128, H, HD], f32, tag="px")
                        for h in range(H):
                            i = h % 6
                            j = (i // 2) + 4 * (i % 2)
                            nc.tensor.matmul(
                                O_ps[:, h, :],
                                e_feat[h // 6][:, j, :],
                                VG[:, tl, h * HD : (h + 1) * HD],
                                start=(h in (0, 8)), stop=(h in (7, 11)),
                            )
                        Osb = pa.tile([128, DIM], bf16, tag="Osb")
                        for h in range(H):
                            i = h % 6
                            nc.vector.tensor_scalar_mul(
                                out=Osb[:, h * HD : (h + 1) * HD],
                                in0=O_ps[:, h, :],
                                scalar1=rden[
                                    :, (h // 6) * 2 + (i % 2), (i // 2) : (i // 2) + 1
                                ],
                            )
                        OTsb = pa.tile([128, CC, 128], bf16, tag="OTsb")
                        OT_ps = px.tile([128, DIM], bf16, tag="px")
                        for cc in range(CC):
                            nc.tensor.transpose(
                                out=OT_ps[:, cc * 128 : (cc + 1) * 128],
                                in_=Osb[:, cc * 128 : (cc + 1) * 128],
                                identity=sb["ident"][:],
                            )
                        nc.vector.tensor_copy(
                            out=OTsb[:], in_=OT_ps[:].rearrange("p (c q) -> p c q", c=CC)
                        )
                        pr_ps = px.tile([128, 2, 512], f32, tag="px")
                        for nh in range(2):
                            for cc in range(CC):
                                nc.tensor.matmul(
                                    pr_ps[:, nh, 0:384],
                                    OTsb[:, cc, :],
                                    wproj[:, cc, nh * 384 : (nh + 1) * 384],
                                    start=(cc == 0), stop=(cc == CC - 1),
                                )
                        att = pa.tile([128, DIM], bf16, tag="att")
                        nc.vector.tensor_tensor(
                            out=att[:].rearrange("p (a n) -> p a n", a=2),
                            in0=pr_ps[:, :, 0:384],
                            in1=sb["bproj"][:].rearrange("p (a n) -> p a n", a=2),
                            op=OP.add,
                        )
                        if t < NW - 1:
                            nc.sync.dma_start(
                                out=attn_scr[x0 + 64 + 128 * t : x0 + 64 + 128 * (t + 1)],
                                in_=att[:],
                            )
                        else:
                            nc.sync.dma_start(
                                out=attn_scr[x0 + 1984 : x0 + 2000], in_=att[0:16, :]
                            )
                            nc.sync.dma_start(out=attn_scr[x0 : x0 + 64], in_=att[64:128, :])

        # =================== PASS B ===================
        if 'B' in passes:
         with tc.tile_pool(name="pb", bufs=2) as pb, \
             tc.tile_pool(name="pb3", bufs=3) as pb3, \
             tc.tile_pool(name="pg", bufs=1) as pg, \
             tc.tile_pool(name="pf", bufs=2, space="PSUM") as pf, \
             tc.tile_pool(name="pt", bufs=2, space="PSUM") as pt, \
             tc.tile_pool(name="pm", bufs=2, space="PSUM") as pm:

            for b in range(BL):
                x0 = b * N
                for g in (1, 2, 3, 0):
                    hT = pb.tile([128, CC, 512], bf16, tag="hT2")
                    mvg = pb.tile([128, NG, 2], f32, tag="mvg2")
                    rstdg = pb.tile([128, NG], f32, tag="rstdg2")
                    x2G = pb.tile([128, NG, DIM], f32, tag="x2G")
                    tss = []
                    for tl in range(4):
                        m = 4 * g + tl
                        ts = min(128, N - 128 * m)
                        tss.append(ts)
                        x_m = pb3.tile([128, DIM], f32, tag="x_m")
                        a_m = pb3.tile([128, DIM], bf16, tag="a_m")
                        nc.sync.dma_start(
                            out=x_m[:ts], in_=p["xs"][x0 + 128 * m : x0 + 128 * m + ts]
                        )
                        nc.sync.dma_start(
                            out=a_m[:ts], in_=attn_scr[x0 + 128 * m : x0 + 128 * m + ts]
                        )
                        if ts < 128:
                            nc.vector.memset(x2G[:, tl, :], 0.0)
                        nc.vector.tensor_add(
                            out=x2G[:ts, tl, :], in0=x_m[:ts], in1=a_m[:ts]
                        )
                        stats = pb3.tile([128, 3, 6], f32, tag="ln_stats2")
                        for sg in range(3):
                            nc.vector.bn_stats(
                                out=stats[:ts, sg, :],
                                in_=x2G[:ts, tl, sg * 256 : (sg + 1) * 256],
                            )
                        if ts < 128:
                            nc.vector.memset(mvg[:, tl, :], 0.0)
                        nc.vector.bn_aggr(out=mvg[:ts, tl, :], in_=stats[:ts])
                    newton_rsqrt(pb3, mvg[:, :, 1], rstdg, "nrb")
                    for tl in range(4):
                        ts = tss[tl]
                        z2 = pb3.tile([128, DIM], bf16, tag="z2")
                        if ts < 128:
                            nc.vector.memset(z2[:], 0.0)
                        nc.vector.tensor_scalar(
                            out=z2[:ts], in0=x2G[:ts, tl, :],
                            scalar1=mvg[:ts, tl, 0:1], scalar2=rstdg[:ts, tl : tl + 1],
                            op0=OP.subtract, op1=OP.mult,
                        )
                        transpose6(pt, z2, hT, tl)

                    gT = pg.tile([128, JB, 512], bf16, tag="gT")
                    for jb in range(JB):
                        f_ps = pf.tile([128, 512], f32, tag="f")
                        for cc in range(CC):
                            nc.tensor.matmul(
                                f_ps[:],
                                wfc1[:, cc, jb * 128 : (jb + 1) * 128],
                                hT[:, cc, :],
                                start=(cc == 0), stop=(cc == CC - 1),
                            )
                        nc.scalar.activation(
                            out=gT[:, jb, :], in_=f_ps[:], func=AF.Gelu,
                            bias=sb["bfc1"][:, jb : jb + 1], scale=1.0,
                        )
                    for tl in range(4):
                        m = 4 * g + tl
                        ts = tss[tl]
                        m_ps = pm.tile([128, 2, 512], f32, tag="m")
                        for nh in range(2):
                            for hc in range(JB):
                                nc.tensor.matmul(
                                    m_ps[:, nh, 0:384],
                                    gT[:, hc, tl * 128 : (tl + 1) * 128],
                                    wfc2[:, hc, nh * 384 : (nh + 1) * 384],
                                    start=(hc == 0), stop=(hc == JB - 1),
                                )
                        o_sb = pb3.tile([128, DIM], f32, tag="o_sb")
                        nc.vector.tensor_tensor(
                            out=o_sb[:ts].rearrange("p (a n) -> p a n", a=2),
                            in0=m_ps[:ts, :, 0:384],
                            in1=x2G[:ts, tl, :].rearrange("p (a n) -> p a n", a=2),
                            op=OP.add,
                        )
                        nc.vector.tensor_add(
                            out=o_sb[:ts], in0=o_sb[:ts], in1=sb["bfc2x"][:ts]
                        )
                        nc.sync.dma_start(
                            out=out_t[x0 + 128 * m : x0 + 128 * m + ts], in_=o_sb[:ts]
                        )

    if fix_waits:
        nsplit = _fix_multi_waits(nc, mybir)
        print(f"_fix_multi_waits: split {nsplit} waits", flush=True)
    return nc


# ---------------------------------------------------------------------------
# host preprocessing
# ---------------------------------------------------------------------------

def _bf(x):
    return np.ascontiguousarray(np.asarray(x, np.float32).astype(BF))


def _precompute(inp):
    qkv_w = np.asarray(inp["qkv_w"], np.float32)
    qkv_b = np.asarray(inp["qkv_b"], np.float32)
    n1w, n1b = np.asarray(inp["norm1_w"], np.float32), np.asarray(inp["norm1_b"], np.float32)
    n2w, n2b = np.asarray(inp["norm2_w"], np.float32), np.asarray(inp["norm2_b"], np.float32)
    proj_w, proj_b = np.asarray(inp["proj_w"], np.float32), np.asarray(inp["proj_b"], np.float32)
    ls1, ls2 = np.asarray(inp["ls1"], np.float32), np.asarray(inp["ls2"], np.float32)
    fc1_w, fc1_b = np.asarray(inp["fc1_w"], np.float32), np.asarray(inp["fc1_b"], np.float32)
    fc2_w, fc2_b = np.asarray(inp["fc2_w"], np.float32), np.asarray(inp["fc2_b"], np.float32)
    rel_bias = np.asarray(inp["rel_bias"], np.float32)

    c = {}
    wqk = _bf(n1w[:, None] * qkv_w[:, : 2 * DIM])           # [768, 1536]
    c["wqk"] = np.ascontiguousarray(wqk.reshape(CC, 128, 2 * DIM).transpose(1, 0, 2))
    wv = _bf(n1w[:, None] * qkv_w[:, 2 * DIM :])
    c["wv"] = np.ascontiguousarray(wv.reshape(CC, 128, DIM).transpose(1, 0, 2))
    qkvb_f = n1b @ qkv_w + qkv_b
    c["bqk"] = np.ascontiguousarray(
        qkvb_f[: 2 * DIM].reshape(12, 128).T.astype(np.float32)
    )
    bv = qkvb_f[2 * DIM :]
    wproj = _bf(proj_w * ls1[None, :])
    c["wproj"] = np.ascontiguousarray(wproj.reshape(CC, 128, DIM).transpose(1, 0, 2))
    c["bproj"] = np.ascontiguousarray(
        np.broadcast_to(((bv @ proj_w + proj_b) * ls1).astype(np.float32), (128, DIM))
    )
    wfc1 = _bf(n2w[:, None] * fc1_w)
    c["wfc1"] = np.ascontiguousarray(wfc1.reshape(CC, 128, MLP).transpose(1, 0, 2))
    c["bfc1"] = np.ascontiguousarray(
        (n2b @ fc1_w + fc1_b).reshape(JB, 128).T.astype(np.float32)
    )
    wfc2 = _bf(fc2_w * ls2[None, :])
    c["wfc2"] = np.ascontiguousarray(wfc2.reshape(JB, 128, DIM).transpose(1, 0, 2))
    c["bfc2x"] = np.ascontiguousarray(
        np.broadcast_to((fc2_b * ls2).astype(np.float32), (128, DIM))
    )

    coords = np.arange(WS)
    rel_idx = coords[None, :] - coords[:, None] + (NPATCH - 1)
    Bmat = rel_bias[rel_idx].transpose(2, 0, 1).astype(np.float32)  # [H, q, k]
    maskrow = np.zeros(WS, np.float32)
    maskrow[16:64] = -30000.0
    Bm = Bmat + maskrow[None, None, :]
    # head order per half: evens then odds (matches S-slot blocks)
    horder = [0, 2, 4, 1, 3, 5]

    def _blocked(mat):  # mat [H, a, b] -> [a, 2, 6, b] exp'd, bf16
        e = np.exp(mat)
        out = np.stack(
            [np.stack([e[6 * half + i] for i in horder], 0) for half in range(2)], 0
        )  # [2, 6, a, b]
        return _bf(out.transpose(2, 0, 1, 3))

    c["btok"] = _blocked(Bmat)
    c["btokm"] = _blocked(Bm)
    c["bfeat"] = _blocked(Bmat.transpose(0, 2, 1))
    c["bfeatm"] = _blocked(Bm.transpose(0, 2, 1))
    c["ident"] = _bf(np.eye(128, dtype=np.float32))
    return c


def kernel(**inputs):
    from concourse.bass_utils import run_bass_kernel_spmd

    if "nc" not in _CACHE:
        _CACHE["nc"] = _build()
    nc = _CACHE["nc"]

    c = _precompute(inputs)
    x = np.asarray(inputs["x"], np.float32)  # [16, 2000, 768]
    in_maps = []
    for core in range(NCORES):
        m = dict(c)
        m["xs"] = np.ascontiguousarray(
            x[core * BL : (core + 1) * BL].reshape(TOK, DIM)
        )
        in_maps.append(m)
    res = run_bass_kernel_spmd(nc, in_maps, core_ids=list(range(NCORES)))
    out = np.stack(
        [res.results[i]["out"].reshape(BL, N, DIM) for i in range(NCORES)]
    ).reshape(B, N, DIM)
    return out.astype(np.float32)


# revision 14
# speedup vs baseline: 1.0154x; 1.0154x over previous
"""Trainium2 Bass kernel for the shifted-window attention block
(nn_Block_6373731467375), SPMD over 8 NeuronCores, data-parallel over batch.

Per core: 2 batch elements. Pass A computes the attention branch in rolled
window space (LN1 folded into qkv weights, dual-S softmax: S token-major for
denominators, S feature-major for the AV matmul), writes the scaled branch
output to a DRAM scratch in original token order. Pass B adds the residual,
applies LN2 (folded into fc1), runs the MLP and writes the final output.
"""

import numpy as np
import ml_dtypes

BF = ml_dtypes.bfloat16

DIM, H, HD, WS, SHIFT, NPATCH, MLP, EPS = 768, 12, 64, 128, 64, 128, 3072, 1e-5
B, N = 16, 2000
NCORES = 8
BL = B // NCORES          # batch elems per core
TOK = BL * N              # 4000
NW = 16                   # rolled 128-token tiles (=windows) per batch elem
NG = 4                    # groups of 4 tiles (512 tokens)
CC = DIM // 128           # 6 contraction chunks
JB = MLP // 128           # 24 hidden blocks
MAGIC = 0x5F3759DF

_CACHE = {}


# ---------------------------------------------------------------------------
# device kernel builder
# ---------------------------------------------------------------------------

def _fix_multi_waits(nc, mybir):
    """This walrus build rejects >1 sync-wait per instruction; hoist extra
    waits onto dedicated NOPs inserted just before, on the same engine."""
    n = 0
    for blk in nc.main_func.blocks:
        new_insts = []
        changed = False
        for ins in blk.instructions:
            si = ins.sync_info
            if si is not None and si.on_wait and len(si.on_wait) > 1:
                waits = list(si.on_wait)
                for w in waits[:-1]:
                    n += 1
                    nop = mybir.InstNoOp(
                        name=f"{ins.name}-sw{n}",
                        engine=ins.engine,
                        ins=[],
                        outs=[],
                        bass_nofuse=True,
                        sync_info=mybir.SyncInfo(on_wait=[w], on_update=[]),
                    )
                    new_insts.append(nop)
                si.on_wait = waits[-1:]
                changed = True
            new_insts.append(ins)
        if changed:
            blk.instructions = new_insts
    return n


def _build(fix_waits=True, passes=('A','B')):
    import concourse.bass as bass
    import concourse.mybir as mybir
    from contextlib import ExitStack

    f32 = mybir.dt.float32
    bf16 = mybir.dt.bfloat16
    u32 = mybir.dt.uint32
    AX = mybir.AxisListType
    OP = mybir.AluOpType
    AF = mybir.ActivationFunctionType

    from concourse.tile import TileContext

    nc = bass.Bass()
    p = {}
    p["xs"] = nc.declare_dram_parameter("xs", [TOK, DIM], f32, isOutput=False)
    p["wqk"] = nc.declare_dram_parameter("wqk", [128, CC, 2 * DIM], bf16, isOutput=False)
    p["wv"] = nc.declare_dram_parameter("wv", [128, CC, DIM], bf16, isOutput=False)
    p["wproj"] = nc.declare_dram_parameter("wproj", [128, CC, DIM], bf16, isOutput=False)
    p["wfc1"] = nc.declare_dram_parameter("wfc1", [128, CC, MLP], bf16, isOutput=False)
    p["wfc2"] = nc.declare_dram_parameter("wfc2", [128, JB, DIM], bf16, isOutput=False)
    p["bqk"] = nc.declare_dram_parameter("bqk", [128, 12], f32, isOutput=False)
    p["bfc1"] = nc.declare_dram_parameter("bfc1", [128, JB], f32, isOutput=False)
    p["bproj"] = nc.declare_dram_parameter("bproj", [128, DIM], f32, isOutput=False)
    p["bfc2x"] = nc.declare_dram_parameter("bfc2x", [128, DIM], f32, isOutput=False)
    p["btok"] = nc.declare_dram_parameter("btok", [128, 2, 6, WS], bf16, isOutput=False)
    p["btokm"] = nc.declare_dram_parameter("btokm", [128, 2, 6, WS], bf16, isOutput=False)
    p["bfeat"] = nc.declare_dram_parameter("bfeat", [128, 2, 6, WS], bf16, isOutput=False)
    p["bfeatm"] = nc.declare_dram_parameter("bfeatm", [128, 2, 6, WS], bf16, isOutput=False)
    p["ident"] = nc.declare_dram_parameter("ident", [128, 128], bf16, isOutput=False)
    out_t = nc.declare_dram_parameter("out", [TOK, DIM], f32, isOutput=True)
    attn_scr = nc.dram_tensor("attn_scr", [TOK, DIM], f32)

    with TileContext(nc) as tc, ExitStack() as ctx:
        cpool = ctx.enter_context(tc.tile_pool(name="consts", bufs=1))

        # resident constants
        sb = {}
        for name in ("bqk", "bfc1", "bproj", "bfc2x", "btok", "btokm",
                     "bfeat", "bfeatm", "ident"):
            t = cpool.tile(list(p[name].shape), p[name].dtype, tag=name)
            nc.sync.dma_start(out=t[:], in_=p[name][:])
            sb[name] = t
        magic = cpool.tile([128, 1], u32, tag="magic")
        nc.vector.memset(magic[:], MAGIC)

        wB = ctx.enter_context(tc.tile_pool(name="wB", bufs=1))
        wfc1 = wB.tile([128, CC, MLP], bf16)
        wfc2 = wB.tile([128, JB, DIM], bf16)

        # ---------------- helpers ----------------
        def newton_rsqrt(pool, var_view, rstdg, tagp):
            """rstdg[:, :NG] = rsqrt(var_view + eps) via 3 fp32 Newton steps."""
            vts = pool.tile([128, NG], f32, tag=tagp + "v")
            y = pool.tile([128, NG], f32, tag=tagp + "y")
            t1 = pool.tile([128, NG], f32, tag=tagp + "t")
            nc.vector.tensor_scalar_add(out=vts[:], in0=var_view, scalar1=EPS)
            nc.vector.tensor_scalar(
                out=y[:].bitcast(u32),
                in0=vts[:].bitcast(u32),
                scalar1=1,
                scalar2=None,
                op0=OP.logical_shift_right,
            )
            nc.vector.tensor_tensor(
                out=y[:].bitcast(u32),
                in0=magic[:].to_broadcast([128, NG]),
                in1=y[:].bitcast(u32),
                op=OP.subtract,
            )
            a, b = y, rstdg
            for _ in range(3):
                nc.vector.tensor_mul(out=t1[:], in0=a[:], in1=a[:])
                nc.vector.tensor_mul(out=t1[:], in0=t1[:], in1=vts[:])
                nc.vector.tensor_scalar(
                    out=t1[:], in0=t1[:], scalar1=-0.5, scalar2=1.5,
                    op0=OP.mult, op1=OP.add,
                )
                nc.vector.tensor_mul(out=b[:], in0=a[:], in1=t1[:])
                a, b = b, a
            assert a is rstdg  # odd iteration count lands in caller's tile

        def transpose6(pool, z_t, dst, tl):
            """z_t [128,768] bf16 -> dst[:, :, tl*128:(tl+1)*128] ([128,6,128])."""
            zT = pool.tile([128, DIM], bf16, tag="px")
            for cc in range(CC):
                nc.tensor.matmul(
                    zT[:, cc * 128 : (cc + 1) * 128],
                    z_t[:, cc * 128 : (cc + 1) * 128],
                    sb["ident"][:],
                    start=(cc == 0), stop=(cc == CC - 1),
                    is_transpose=True,
                )
            nc.vector.tensor_copy(
                out=dst[:, :, tl * 128 : (tl + 1) * 128],
                in_=zT[:].rearrange("p (c q) -> p c q", c=CC),
            )

        # =================== PASS A ===================
        if 'A' in passes:
         with tc.tile_pool(name="wA", bufs=1) as wA, \
             tc.tile_pool(name="pa", bufs=2) as pa, \
             tc.tile_pool(name="pa1", bufs=1) as pa1, \
             tc.tile_pool(name="pa3", bufs=3) as pa3, \
             tc.tile_pool(name="px", bufs=2, space="PSUM") as px, \
             tc.tile_pool(name="ps", bufs=2, space="PSUM") as ps:

            wqk = wA.tile([128, CC, 2 * DIM], bf16)
            nc.sync.dma_start(out=wqk[:], in_=p["wqk"][:])
            wv = wA.tile([128, CC, DIM], bf16)
            nc.sync.dma_start(out=wv[:], in_=p["wv"][:])
            wproj = wA.tile([128, CC, DIM], bf16)
            nc.sync.dma_start(out=wproj[:], in_=p["wproj"][:])

            for b in range(BL):
                x0 = b * N
                for g in range(NG):
                    if b == 0 and g == 1:
                        # prefetch MLP weights once the startup DMA burst clears
                        nc.gpsimd.dma_start(out=wfc1[:], in_=p["wfc1"][:])
                        nc.gpsimd.dma_start(out=wfc2[:], in_=p["wfc2"][:])
                    hT = pa.tile([128, CC, 512], bf16, tag="hT")
                    mvg = pa.tile([128, NG, 2], f32, tag="mvg")
                    rstdg = pa.tile([128, NG], f32, tag="rstdg")
                    xG = pa.tile([128, NG, DIM], bf16, tag="xG")
                    for tl in range(4):
                        t = 4 * g + tl
                        if t < NW - 1:
                            nc.gpsimd.dma_start(
                                out=xG[:, tl, :],
                                in_=p["xs"][x0 + 64 + 128 * t : x0 + 64 + 128 * (t + 1)],
                            )
                        else:
                            nc.vector.memset(xG[:, tl, :], 0.0)
                            nc.gpsimd.dma_start(
                                out=xG[0:16, tl, :], in_=p["xs"][x0 + 1984 : x0 + 2000]
                            )
                            nc.gpsimd.dma_start(
                                out=xG[64:128, tl, :], in_=p["xs"][x0 : x0 + 64]
                            )
                        stats = pa3.tile([128, 3, 6], f32, tag="ln_stats")
                        for sg in range(3):
                            nc.vector.bn_stats(
                                out=stats[:, sg, :],
                                in_=xG[:, tl, sg * 256 : (sg + 1) * 256],
                            )
                        nc.vector.bn_aggr(out=mvg[:, tl, :], in_=stats[:])
                    newton_rsqrt(pa3, mvg[:, :, 1], rstdg, "nra")
                    for tl in range(4):
                        z_t = pa3.tile([128, DIM], bf16, tag="z_t")
                        nc.vector.tensor_scalar(
                            out=z_t[:], in0=xG[:, tl, :],
                            scalar1=mvg[:, tl, 0:1], scalar2=rstdg[:, tl : tl + 1],
                            op0=OP.subtract, op1=OP.mult,
                        )
                        transpose6(px, z_t, hT, tl)

                    # qkv for the group
                    QKT = pa.tile([128, 12, 512], bf16, tag="QKT")
                    for fb in range(12):
                        qk_ps = px.tile([128, 512], f32, tag="px")
                        for cc in range(CC):
                            nc.tensor.matmul(
                                qk_ps[:],
                                wqk[:, cc, fb * 128 : (fb + 1) * 128],
                                hT[:, cc, :],
                                start=(cc == 0), stop=(cc == CC - 1),
                            )
                        nc.vector.tensor_scalar(
                            out=QKT[:, fb, :], in0=qk_ps[:],
                            scalar1=sb["bqk"][:, fb : fb + 1],
                            scalar2=(0.125 if fb < 6 else 1.0),
                            op0=OP.add, op1=OP.mult,
                        )
                    VG = pa1.tile([128, 4, DIM], bf16, tag="VG")
                    for tl in range(4):
                        v_ps = px.tile([128, 2, 512], f32, tag="px")
                        for cc in range(CC):
                            for nh in range(2):
                                nc.tensor.matmul(
                                    v_ps[:, nh, 0:384],
                                    hT[:, cc, tl * 128 : (tl + 1) * 128],
                                    wv[:, cc, nh * 384 : (nh + 1) * 384],
                                    start=(cc == 0), stop=(cc == CC - 1),
                                )
                        nc.vector.tensor_copy(
                            out=VG[:, tl, :].rearrange("p (a n) -> p a n", a=2),
                            in_=v_ps[:, :, 0:384],
                        )

                    # windows
                    for tl in range(4):
                        t = 4 * g + tl
                        masked = t == NW - 1
                        btok_t = sb["btokm"] if masked else sb["btok"]
                        bfeat_t = sb["bfeatm"] if masked else sb["bfeat"]
                        qs = slice(tl * 128, (tl + 1) * 128)

                        # Even/odd heads target different PSUM banks: MMs with
                        # disjoint PE row-groups (base partition 0 vs 64) run
                        # concurrently, and concurrent writes to one PSUM bank
                        # hard-fault the device. Slot j: even i -> i//2 (bank
                        # 0), odd i -> 4 + i//2 (bank 1).
                        den = pa.tile([128, 4, 3], f32, tag="den")
                        rden = pa.tile([128, 4, 3], f32, tag="rden")
                        e_feat = []
                        for half in range(2):
                            hh = list(range(half * 6, half * 6 + 6))

                            def _smm(s_t, lhs_fb, rhs_fb):
                                for i, h in enumerate(hh):
                                    bp = (h % 2) * 64
                                    j = (i // 2) + 4 * (i % 2)
                                    nc.tensor.matmul(
                                        s_t[:, j, :],
                                        QKT[bp : bp + 64, lhs_fb + h // 2, qs],
                                        QKT[bp : bp + 64, rhs_fb + h // 2, qs],
                                        start=(i in (0, 1)), stop=(i in (4, 5)),
                                    )

                            s_tok = ps.tile([128, 8, 128], f32, tag="s")
                            _smm(s_tok, 0, 6)
                            E_tok = pa.tile([128, 8, 128], bf16, tag="E_tok")
                            for blk in range(2):
                                js = slice(4 * blk, 4 * blk + 3)
                                nc.scalar.activation(
                                    out=E_tok[:, js, :], in_=s_tok[:, js, :],
                                    func=AF.Exp,
                                )
                                nc.vector.tensor_mul(
                                    out=E_tok[:, js, :], in0=E_tok[:, js, :],
                                    in1=btok_t[:, half, 3 * blk : 3 * blk + 3, :],
                                )
                                nc.vector.tensor_reduce(
                                    out=den[:, half * 2 + blk, :],
                                    in_=E_tok[:, js, :],
                                    axis=AX.X, op=OP.add,
                                )

                            s_feat = ps.tile([128, 8, 128], f32, tag="s")
                            _smm(s_feat, 6, 0)
                            E_f = pa.tile([128, 8, 128], bf16, tag="E_feat")
                            for blk in range(2):
                                js = slice(4 * blk, 4 * blk + 3)
                                nc.scalar.activation(
                                    out=E_f[:, js, :], in_=s_feat[:, js, :],
                                    func=AF.Exp,
                                )
                                nc.vector.tensor_mul(
                                    out=E_f[:, js, :], in0=E_f[:, js, :],
                                    in1=bfeat_t[:, half, 3 * blk : 3 * blk + 3, :],
                                )
                            e_feat.append(E_f)
                        nc.vector.reciprocal(out=rden[:], in_=den[:])

                        O_ps = px.tile([128, H, HD], f32, tag="px")
                        for h in range(H):
                            i = h % 6
                            j = (i // 2) + 4 * (i % 2)
                            nc.tensor.matmul(
                                O_ps[:, h, :],
                                e_feat[h // 6][:, j, :],
                                VG[:, tl, h * HD : (h + 1) * HD],
                                start=(h in (0, 8)), stop=(h in (7, 11)),
                            )
                        Osb = pa.tile([128, DIM], bf16, tag="Osb")
                        for h in range(H):
                            i = h % 6
                            nc.vector.tensor_scalar_mul(
                                out=Osb[:, h * HD : (h + 1) * HD],
                                in0=O_ps[:, h, :],
                                scalar1=rden[
                                    :, (h // 6) * 2 + (i % 2), (i // 2) : (i // 2) + 1
                                ],
                            )
                        OTsb = pa.tile([128, CC, 128], bf16, tag="OTsb")
                        OT_ps = px.tile([128, DIM], bf16, tag="px")
                        for cc in range(CC):
                            nc.tensor.transpose(
                                out=OT_ps[:, cc * 128 : (cc + 1) * 128],
                                in_=Osb[:, cc * 128 : (cc + 1) * 128],
                                identity=sb["ident"][:],
                            )
                        nc.vector.tensor_copy(
                            out=OTsb[:], in_=OT_ps[:].rearrange("p (c q) -> p c q", c=CC)
                        )
                        pr_ps = px.tile([128, 2, 512], f32, tag="px")
                        for cc in range(CC):
                            for nh in range(2):
                                nc.tensor.matmul(
                                    pr_ps[:, nh, 0:384],
                                    OTsb[:, cc, :],
                                    wproj[:, cc, nh * 384 : (nh + 1) * 384],
                                    start=(cc == 0), stop=(cc == CC - 1),
                                )
                        att = pa.tile([128, DIM], f32, tag="att")
                        nc.vector.tensor_tensor(
                            out=att[:].rearrange("p (a n) -> p a n", a=2),
                            in0=pr_ps[:, :, 0:384],
                            in1=sb["bproj"][:].rearrange("p (a n) -> p a n", a=2),
                            op=OP.add,
                        )
                        if t < NW - 1:
                            nc.sync.dma_start(
                                out=attn_scr[x0 + 64 + 128 * t : x0 + 64 + 128 * (t + 1)],
                                in_=att[:],
                            )
                        else:
                            nc.sync.dma_start(
                                out=attn_scr[x0 + 1984 : x0 + 2000], in_=att[0:16, :]
                            )
                            nc.sync.dma_start(out=attn_scr[x0 : x0 + 64], in_=att[64:128, :])

        # =================== PASS B ===================
        if 'B' in passes:
         with tc.tile_pool(name="pb", bufs=2) as pb, \
             tc.tile_pool(name="pb3", bufs=3) as pb3, \
             tc.tile_pool(name="pg", bufs=1) as pg, \
             tc.tile_pool(name="pf", bufs=2, space="PSUM") as pf, \
             tc.tile_pool(name="pt", bufs=2, space="PSUM") as pt, \
             tc.tile_pool(name="pm", bufs=2, space="PSUM") as pm:

            for b in range(BL):
                x0 = b * N
                for g in (1, 2, 3, 0):
                    hT = pb.tile([128, CC, 512], bf16, tag="hT2")
                    mvg = pb.tile([128, NG, 2], f32, tag="mvg2")
                    rstdg = pb.tile([128, NG], f32, tag="rstdg2")
                    x2G = pb.tile([128, NG, DIM], f32, tag="x2G")
                    tss = []
                    for tl in range(4):
                        m = 4 * g + tl
                        ts = min(128, N - 128 * m)
                        tss.append(ts)
                        x_m = pb3.tile([128, DIM], f32, tag="x_m")
                        a_m = pb3.tile([128, DIM], f32, tag="a_m")
                        nc.sync.dma_start(
                            out=x_m[:ts], in_=p["xs"][x0 + 128 * m : x0 + 128 * m + ts]
                        )
                        nc.sync.dma_start(
                            out=a_m[:ts], in_=attn_scr[x0 + 128 * m : x0 + 128 * m + ts]
                        )
                        if ts < 128:
                            nc.vector.memset(x2G[:, tl, :], 0.0)
                        nc.vector.tensor_add(
                            out=x2G[:ts, tl, :], in0=x_m[:ts], in1=a_m[:ts]
                        )
                        stats = pb3.tile([128, 3, 6], f32, tag="ln_stats2")
                        for sg in range(3):
                            nc.vector.bn_stats(
                                out=stats[:ts, sg, :],
                                in_=x2G[:ts, tl, sg * 256 : (sg + 1) * 256],
                            )
                        if ts < 128:
                            nc.vector.memset(mvg[:, tl, :], 0.0)
                        nc.vector.bn_aggr(out=mvg[:ts, tl, :], in_=stats[:ts])
                    newton_rsqrt(pb3, mvg[:, :, 1], rstdg, "nrb")
                    for tl in range(4):
                        ts = tss[tl]
                        z2 = pb3.tile([128, DIM], bf16, tag="z2")
                        if ts < 128:
                            nc.vector.memset(z2[:], 0.0)
                        nc.vector.tensor_scalar(
                            out=z2[:ts], in0=x2G[:ts, tl, :],
                            scalar1=mvg[:ts, tl, 0:1], scalar2=rstdg[:ts, tl : tl + 1],
                            op0=OP.subtract, op1=OP.mult,
                        )
                        transpose6(pt, z2, hT, tl)

                    gT = pg.tile([128, JB, 512], bf16, tag="gT")
                    for jb in range(JB):
                        f_ps = pf.tile([128, 512], f32, tag="f")
                        for cc in range(CC):
                            nc.tensor.matmul(
                                f_ps[:],
                                wfc1[:, cc, jb * 128 : (jb + 1) * 128],
                                hT[:, cc, :],
                                start=(cc == 0), stop=(cc == CC - 1),
                            )
                        nc.scalar.activation(
                            out=gT[:, jb, :], in_=f_ps[:], func=AF.Gelu,
                            bias=sb["bfc1"][:, jb : jb + 1], scale=1.0,
                        )
                    for tl in range(4):
                        m = 4 * g + tl
                        ts = tss[tl]
                        m_ps = pm.tile([128, 2, 512], f32, tag="m")
                        for hc in range(JB):
                            for nh in range(2):
                                nc.tensor.matmul(
                                    m_ps[:, nh, 0:384],
                                    gT[:, hc, tl * 128 : (tl + 1) * 128],
                                    wfc2[:, hc, nh * 384 : (nh + 1) * 384],
                                    start=(hc == 0), stop=(hc == JB - 1),
                                )
                        o_sb = pb3.tile([128, DIM], f32, tag="o_sb")
                        nc.vector.tensor_tensor(
                            out=o_sb[:ts].rearrange("p (a n) -> p a n", a=2),
                            in0=m_ps[:ts, :, 0:384],
                            in1=x2G[:ts, tl, :].rearrange("p (a n) -> p a n", a=2),
                            op=OP.add,
                        )
                        nc.vector.tensor_add(
                            out=o_sb[:ts], in0=o_sb[:ts], in1=sb["bfc2x"][:ts]
                        )
                        nc.sync.dma_start(
                            out=out_t[x0 + 128 * m : x0 + 128 * m + ts], in_=o_sb[:ts]
                        )

    if fix_waits:
        nsplit = _fix_multi_waits(nc, mybir)
        print(f"_fix_multi_waits: split {nsplit} waits", flush=True)
    return nc


# ---------------------------------------------------------------------------
# host preprocessing
# ---------------------------------------------------------------------------

def _bf(x):
    return np.ascontiguousarray(np.asarray(x, np.float32).astype(BF))


def _precompute(inp):
    qkv_w = np.asarray(inp["qkv_w"], np.float32)
    qkv_b = np.asarray(inp["qkv_b"], np.float32)
    n1w, n1b = np.asarray(inp["norm1_w"], np.float32), np.asarray(inp["norm1_b"], np.float32)
    n2w, n2b = np.asarray(inp["norm2_w"], np.float32), np.asarray(inp["norm2_b"], np.float32)
    proj_w, proj_b = np.asarray(inp["proj_w"], np.float32), np.asarray(inp["proj_b"], np.float32)
    ls1, ls2 = np.asarray(inp["ls1"], np.float32), np.asarray(inp["ls2"], np.float32)
    fc1_w, fc1_b = np.asarray(inp["fc1_w"], np.float32), np.asarray(inp["fc1_b"], np.float32)
    fc2_w, fc2_b = np.asarray(inp["fc2_w"], np.float32), np.asarray(inp["fc2_b"], np.float32)
    rel_bias = np.asarray(inp["rel_bias"], np.float32)

    c = {}
    wqk = _bf(n1w[:, None] * qkv_w[:, : 2 * DIM])           # [768, 1536]
    c["wqk"] = np.ascontiguousarray(wqk.reshape(CC, 128, 2 * DIM).transpose(1, 0, 2))
    wv = _bf(n1w[:, None] * qkv_w[:, 2 * DIM :])
    c["wv"] = np.ascontiguousarray(wv.reshape(CC, 128, DIM).transpose(1, 0, 2))
    qkvb_f = n1b @ qkv_w + qkv_b
    c["bqk"] = np.ascontiguousarray(
        qkvb_f[: 2 * DIM].reshape(12, 128).T.astype(np.float32)
    )
    bv = qkvb_f[2 * DIM :]
    wproj = _bf(proj_w * ls1[None, :])
    c["wproj"] = np.ascontiguousarray(wproj.reshape(CC, 128, DIM).transpose(1, 0, 2))
    c["bproj"] = np.ascontiguousarray(
        np.broadcast_to(((bv @ proj_w + proj_b) * ls1).astype(np.float32), (128, DIM))
    )
    wfc1 = _bf(n2w[:, None] * fc1_w)
    c["wfc1"] = np.ascontiguousarray(wfc1.reshape(CC, 128, MLP).transpose(1, 0, 2))
    c["bfc1"] = np.ascontiguousarray(
        (n2b @ fc1_w + fc1_b).reshape(JB, 128).T.astype(np.float32)
    )
    wfc2 = _bf(fc2_w * ls2[None, :])
    c["wfc2"] = np.ascontiguousarray(wfc2.reshape(JB, 128, DIM).transpose(1, 0, 2))
    c["bfc2x"] = np.ascontiguousarray(
        np.broadcast_to((fc2_b * ls2).astype(np.float32), (128, DIM))
    )

    coords = np.arange(WS)
    rel_idx = coords[None, :] - coords[:, None] + (NPATCH - 1)
    Bmat = rel_bias[rel_idx].transpose(2, 0, 1).astype(np.float32)  # [H, q, k]
    maskrow = np.zeros(WS, np.float32)
    maskrow[16:64] = -30000.0
    Bm = Bmat + maskrow[None, None, :]
    # head order per half: evens then odds (matches S-slot blocks)
    horder = [0, 2, 4, 1, 3, 5]

    def _blocked(mat):  # mat [H, a, b] -> [a, 2, 6, b] exp'd, bf16
        e = np.exp(mat)
        out = np.stack(
            [np.stack([e[6 * half + i] for i in horder], 0) for half in range(2)], 0
        )  # [2, 6, a, b]
        return _bf(out.transpose(2, 0, 1, 3))

    c["btok"] = _blocked(Bmat)
    c["btokm"] = _blocked(Bm)
    c["bfeat"] = _blocked(Bmat.transpose(0, 2, 1))
    c["bfeatm"] = _blocked(Bm.transpose(0, 2, 1))
    c["ident"] = _bf(np.eye(128, dtype=np.float32))
    return c


def kernel(**inputs):
    from concourse.bass_utils import run_bass_kernel_spmd

    if "nc" not in _CACHE:
        _CACHE["nc"] = _build()
    nc = _CACHE["nc"]

    c = _precompute(inputs)
    x = np.asarray(inputs["x"], np.float32)  # [16, 2000, 768]
    in_maps = []
    for core in range(NCORES):
        m = dict(c)
        m["xs"] = np.ascontiguousarray(
            x[core * BL : (core + 1) * BL].reshape(TOK, DIM)
        )
        in_maps.append(m)
    res = run_bass_kernel_spmd(nc, in_maps, core_ids=list(range(NCORES)))
    out = np.stack(
        [res.results[i]["out"].reshape(BL, N, DIM) for i in range(NCORES)]
    ).reshape(B, N, DIM)
    return out.astype(np.float32)
